# revision 11
# baseline (speedup 1.0000x reference)
"""Embedding-lookup kernel for Trainium2 (8 NeuronCores).

Problem: out[b, t, :] = W[batch_positions[b, t], :] + bias
  batch_positions: [8192, 64] int, values in [0, 365)
  W: [365, 128] f32, bias: [128] f32
  out: [8192, 64, 128] f32

Strategy (data-parallel over batch, 8 cores):
  * bias folded into W host-side (W2 = W + bias) -- identical f32
    arithmetic to the reference (gather-then-add == add-then-gather).
  * per core: 1024 batch rows = 65536 indices.
  * device kernel: all int16 index tiles preloaded into SBUF in one DMA,
    then a pipeline of dma_gather calls (SWDGE embedding gather,
    HBM -> SBUF, 512B rows, rotating over 4 SWDGE queues, 8 slots deep)
    with large contiguous SBUF -> HBM writes.  Indices are pre-permuted
    host-side so SBUF partition p holds a contiguous run of output rows,
    which makes each write-side DMA descriptor a 4KB contiguous run.

Measured (paired-difference device timing, 8 cores concurrent):
  ~220 us per invocation, bit-exact vs the f32 reference.
  HBM R+W floor for this design ~187 us; 33.5 MB/core output.
Tuning history: nbuf=2/q=1 634us -> q=4 564us -> nbuf=4 368us ->
  nbuf=8 238us -> idx preload ~220us.  G=2048/call wedges the device
  (SWDGE ring capacity); pairing writes or adding the ACT HWDGE engine
  made things worse/unstable.
"""

from contextlib import ExitStack

import numpy as np

import concourse.mybir as mybir
from concourse import bacc, bass_utils, library_config
from concourse._compat import get_trn_type

# ---- problem shapes (hardcoded; kernel.py must be self-contained) ----
B, T, D = 8192, 64, 128
NUM_DAYS = 365
N_CORES = 8
ROWS_PER_CORE = B // N_CORES            # 1024
N_IDX = ROWS_PER_CORE * T               # 65536 indices per core

# ---- tunables ----
G = 1024                                # indices per dma_gather call
NCALLS = N_IDX // G
NBLK = G // 128                         # output rows per partition per call
G16 = G // 16                           # idx tile free dim (16-wrap)
NBUF = 8                                # pipeline depth (gather buffers)
NQUEUES = 4                             # SWDGE queues (2 Q7 cores each)

_cache = {}


def _build_bass(reps=1):
    """Per-core program (same NEFF on all 8 cores, different data)."""
    nc = bacc.Bacc(get_trn_type() or "TRN2", num_swdge_queues=NQUEUES)

    idx_l = nc.dram_tensor("idx_l", [NCALLS, 128, G16], mybir.dt.int16,
                           kind="ExternalInput")
    w = nc.dram_tensor("w", [NUM_DAYS, D], mybir.dt.float32,
                       kind="ExternalInput")
    out = nc.dram_tensor("out", [N_IDX, D], mybir.dt.float32,
                         kind="ExternalOutput")

    with ExitStack() as ctx:
        idx_sb = ctx.enter_context(
            nc.sbuf_tensor("idx_sb", [128, NCALLS, G16], mybir.dt.int16))
        g_sb = ctx.enter_context(
            nc.sbuf_tensor("g_sb", [128, NBUF, NBLK, D], mybir.dt.float32))
        sem_idx = ctx.enter_context(nc.semaphore(name="sem_idx"))
        # per-slot semaphores: at most one in-flight incrementer per sem,
        # so "wait >= 16*k" is race-free.
        sem_g = [ctx.enter_context(nc.semaphore(name=f"sem_g{i}"))
                 for i in range(NBUF)]
        sem_out = [ctx.enter_context(nc.semaphore(name=f"sem_out{i}"))
                   for i in range(NBUF)]
        block = ctx.enter_context(nc.Block())

        total = reps * NCALLS

        @block.sync
        def _(sync):
            # one preload of every idx tile ([t, p, :] -> [p, t, :])
            sync.dma_start(idx_sb[:],
                           idx_l[:].rearrange("t p g -> p t g")
                           ).then_inc(sem_idx, 16)
            for t in range(total):
                s, k = t % NBUF, t // NBUF
                sync.wait_ge(sem_g[s], 16 * (k + 1))
                tc = t % NCALLS
                out_ap = out[tc * G:(tc + 1) * G].rearrange(
                    "(p blk) d -> p blk d", p=128)
                sync.dma_start(out_ap, g_sb[:, s]).then_inc(sem_out[s], 16)
            for s in range(NBUF):
                n = total // NBUF + (total % NBUF > s)
                if n:
                    sync.wait_ge(sem_out[s], 16 * n)

        @block.gpsimd
        def _(gpsimd):
            gpsimd.load_library(library_config.mlp)
            gpsimd.wait_ge(sem_idx, 16)
            for t in range(total):
                s, k = t % NBUF, t // NBUF
                if t >= NBUF:
                    # g slot s free once writeout t-NBUF (same slot) done
                    gpsimd.wait_ge(sem_out[s], 16 * k)
                gpsimd.dma_gather(
                    g_sb[:, s], w[:, :], idx_sb[:, t % NCALLS],
                    num_idxs=G, num_idxs_reg=G, elem_size=D,
                    queue_num=t % NQUEUES,
                ).then_inc(sem_g[s], 16)

    nc.compile()
    return nc


def _prep_idx(idx_core: np.ndarray) -> np.ndarray:
    """[N_IDX] int -> [NCALLS, 128, G16] int16 in dma_gather layout.

    Within call t, fed[blk*128 + p] = orig[p*NBLK + blk] so the gathered
    row for output position p*NBLK+blk lands at partition p, block blk
    (contiguous DRAM run per partition on the write side).  The
    instruction reads indices 16-partition-wrapped, replicated 8x
    across the 128 partitions.
    """
    idx3 = idx_core.reshape(NCALLS, 128, NBLK).astype(np.int16)
    fed = idx3.transpose(0, 2, 1).reshape(NCALLS, G)
    wrap = fed.reshape(NCALLS, G16, 16).transpose(0, 2, 1)
    return np.ascontiguousarray(np.tile(wrap, (1, 8, 1)))


def _run(batch_positions, W, b, trace=False):
    if "nc" not in _cache:
        _cache["nc"] = _build_bass()
    nc = _cache["nc"]

    w2 = (np.asarray(W, dtype=np.float32)
          + np.asarray(b, dtype=np.float32)[None, :])
    idx = np.asarray(batch_positions).reshape(B, T)

    in_maps = []
    for c in range(N_CORES):
        idx_core = idx[c * ROWS_PER_CORE:(c + 1) * ROWS_PER_CORE].reshape(-1)
        in_maps.append({"idx_l": _prep_idx(idx_core), "w": w2})

    res = bass_utils.run_bass_kernel_spmd(
        nc, in_maps, core_ids=list(range(N_CORES)), trace=trace)

    out = np.empty((B, T, D), dtype=np.float32)
    for c in range(N_CORES):
        out[c * ROWS_PER_CORE:(c + 1) * ROWS_PER_CORE] = (
            res.results[c]["out"].reshape(ROWS_PER_CORE, T, D))
    return out, res


def kernel(**inputs) -> np.ndarray:
    out, _ = _run(inputs["batch_positions"], inputs["W"], inputs["b"])
    return out
